# revision 3
# baseline (speedup 1.0000x reference)
"""Based-style linear attention (Taylor feature map) on 8 Trainium2 cores.

Math: reference computes, per head h (FDIM=16, HEAD_DIM=64):
    q,k = HS@Wq, HS@Wk    (per-head 16 dims), v = HS@Wv (per-head 64 dims)
    phi(x) = [1, x/2, outer(x,x)/(sqrt(2)*4)]      (273 dims)
    y_t = sum_{s<=t} (phi(q_t).phi(k_s)) v_s / sum_{s<=t} phi(q_t).phi(k_s)
    out = concat_h(y) @ Wo
Key identity: phi(q).phi(k) = Square(S/sqrt(32) + 1/sqrt(2)) + 1/2, S = q.k.

Sharding: head-parallel, no collectives. 16 virtual heads (12 real + 4
zero dummies), 2 per core. Host sums the 8 partial outputs.

v2 (vs v1 baseline, 66.6us):
 - PE warmup: dummy matmuls during the input-DMA wait keep the HAM clock
   gate at K=8/8 (2.4 GHz). v1 ran ~36us of its span throttled at 1.2.
 - Streaming finalization: after kv-chunk j the num columns for t-chunk j
   are complete, so recip/div + o-proj + output DMA for chunk j run
   interleaved with scores of chunks j+1.. (software-pipelined one chunk
   behind to avoid head-of-line blocking the PE queue on DVE work).
   v1 did all 8 o-proj chunks + DMAs in a serial tail.
 - 7 input DMA issues over both HWDGE queues (sync+scalar) instead of 15
   serialized on sync.
"""

import math

import numpy as np
import ml_dtypes

import concourse.bass as bass
import concourse.mybir as mybir
import concourse.tile as tile
from concourse import bacc
from concourse.bass_utils import run_bass_kernel_spmd

L = 1024
D = 768
H = 12
FD = 16
HD = 64
NCORE = 8
NCH = 8  # L chunks of 128
KB = 6  # contraction blocks of 128 over D
F32 = mybir.dt.float32
BF16 = mybir.dt.bfloat16

DT_PROJ = BF16
DT_ATT = BF16
DT_OUT = BF16

A_SCALE = 1.0 / math.sqrt(32.0)
A_BIAS = 1.0 / math.sqrt(2.0)

N_WARM = 10  # dummy matmuls (N=512) to trip the HAM clock gate to 8/8

_compiled_nc = None
_last_in_maps = None


def _np_dt(dt):
    return ml_dtypes.bfloat16 if dt == BF16 else np.float32


def _bank_splits(lo, hi, bank=512):
    """Split [lo, hi) at multiples of `bank` (PSUM bank boundaries)."""
    out = []
    a = lo
    while a < hi:
        b = min(hi, (a // bank + 1) * bank)
        out.append((a, b))
        a = b
    return out


def _build_nc():
    nc = bacc.Bacc("TRN2", target_bir_lowering=False, debug=False, num_devices=NCORE)

    hsT = nc.dram_tensor("hsT", [D, L], DT_PROJ, kind="ExternalInput")
    wqv = nc.dram_tensor("wqv", [D, 258], DT_PROJ, kind="ExternalInput")
    wo = nc.dram_tensor("wo", [128, D], DT_OUT, kind="ExternalInput")
    # consts packed: tri 0:128 | htri 128:256 | ones8 256:320 | sel 320:1344
    c_all = nc.dram_tensor("c_all", [128, 1344], DT_ATT, kind="ExternalInput")
    out = nc.dram_tensor("out", [L, D], DT_OUT, kind="ExternalOutput")

    with tile.TileContext(nc) as tc:
        with (
            tc.tile_pool(name="cst", bufs=1) as cst,
            tc.tile_pool(name="sqp", bufs=4) as sqp,
            tc.tile_pool(name="wrk", bufs=2) as wrk,
        ):
            # ---- warmup fodder (no data deps; scheduled first) ----
            warm_st = cst.tile([1, 128], DT_ATT, tag="warm_st")
            warm_mv = cst.tile([1, 512], DT_ATT, tag="warm_mv")
            nc.vector.memset(warm_st, 0.0)
            nc.vector.memset(warm_mv, 0.0)

            # ---- input DMAs: hs on sync queue, weights/consts on scalar ----
            wqv_re = wqv.ap().rearrange("(po pi) f -> pi po f", pi=128)
            hs_re = hsT.ap().rearrange("(po pi) f -> pi po f", pi=128)
            hs_sb = cst.tile([128, KB, L], DT_PROJ, tag="hs")
            nc.sync.dma_start(out=hs_sb[:, 0:1, :], in_=hs_re[:, 0:1, :])
            nc.sync.dma_start(out=hs_sb[:, 1:3, :], in_=hs_re[:, 1:3, :])
            nc.sync.dma_start(out=hs_sb[:, 3:6, :], in_=hs_re[:, 3:6, :])
            wqv_sb = cst.tile([128, KB, 258], DT_PROJ, tag="wqv")
            nc.scalar.dma_start(out=wqv_sb, in_=wqv_re)
            wk_sb = wqv_sb[:, :, 0:64]
            wq_sb = wqv_sb[:, :, 64:128]
            wv_sb = wqv_sb[:, :, 128:258]
            call_sb = cst.tile([128, 1344], DT_ATT, tag="call")
            nc.scalar.dma_start(out=call_sb, in_=c_all.ap())
            tri_sb = call_sb[:, 0:128]
            htri_sb = call_sb[:, 128:256]
            ones8_sb = call_sb[:, 256:320]
            sel_sb = call_sb[0:8, 320:1344]
            # wo as two base-0 tiles (o-proj rhs must share base with lhsT)
            wo_sb = []
            for h in range(2):
                t = cst.tile([64, D], DT_OUT, tag=f"wo{h}", name=f"wo{h}")
                nc.scalar.dma_start(out=t, in_=wo.ap()[64 * h : 64 * h + 64, :])
                wo_sb.append(t)
            bias_sb = cst.tile([128, 1], F32, tag="bias")
            nc.vector.memset(bias_sb, A_BIAS)
            # row of ones at partition 64, for the den-reciprocal broadcast
            ones64_sb = cst.tile([65, 64], F32, tag="ones64")
            nc.vector.memset(ones64_sb, 0.0)
            nc.vector.memset(ones64_sb[64:65, :], 1.0)

            kq_sb = cst.tile([64, 2048], DT_ATT, tag="kq")
            vx_sb = cst.tile([128, NCH, 130], DT_ATT, tag="vx")
            colsum_sb = cst.tile([8, 130], DT_ATT, tag="colsum")

            # ================= warmup + projections =================
            with tc.tile_pool(name="psw", bufs=1, space="PSUM") as psw:
                pwarm = psw.tile([128, 512], F32, tag="pw", name="pwarm")
                with tc.high_priority():
                    for i in range(N_WARM):
                        nc.tensor.matmul(pwarm, warm_st, warm_mv, start=True, stop=True)

                with tc.tile_pool(name="ps1", bufs=4, space="PSUM") as ps1:
                    # q/k -> kq_sb [64, 2048]; partitions 0-15 head0, 32-47
                    # head1 (rest zero); cols 0-1023 = k^T, 1024-2047 = q^T
                    pqk = {}
                    for half in range(2):
                        for w_sb, coff in ((wk_sb, 0), (wq_sb, 1024)):
                            pqk[(half, coff)] = ps1.tile(
                                [64, 512], F32, tag="pB", name=f"pqk{coff}_{half}"
                            )
                    for kb in range(KB):
                        for half in range(2):
                            for w_sb, coff in ((wk_sb, 0), (wq_sb, 1024)):
                                nc.tensor.matmul(
                                    pqk[(half, coff)],
                                    w_sb[:, kb, :],
                                    hs_sb[:, kb, half * 512 : (half + 1) * 512],
                                    start=(kb == 0),
                                    stop=(kb == KB - 1),
                                )
                    for half in range(2):
                        for coff in (0, 1024):
                            nc.vector.tensor_copy(
                                kq_sb[:, coff + half * 512 : coff + (half + 1) * 512],
                                pqk[(half, coff)],
                            )

                # v -> vx_sb [128, 8, 130]: cols 0-63 v_h0, 64 ones,
                # 65-128 v_h1, 129 ones
                with tc.tile_pool(name="psv", bufs=3, space="PSUM") as psv:
                    for ch in range(NCH):
                        pv = psv.tile([128, 130], F32, tag="pB", name=f"pv{ch}")
                        for kb in range(KB):
                            nc.tensor.matmul(
                                pv,
                                hs_sb[:, kb, ch * 128 : (ch + 1) * 128],
                                wv_sb[:, kb, :],
                                start=(kb == 0),
                                stop=(kb == KB - 1),
                            )
                        nc.vector.tensor_copy(vx_sb[:, ch, :], pv)
                    nc.vector.memset(vx_sb[:, :, 64], 1.0)
                    nc.vector.memset(vx_sb[:, :, 129], 1.0)

                    # per-chunk column sums of vx (inter-chunk +1/2 term)
                    pcs = psv.tile([8, 130], F32, tag="pB", name="pcs")
                    for ch in range(NCH):
                        nc.tensor.matmul(
                            pcs,
                            ones8_sb[:, ch * 8 : (ch + 1) * 8],
                            vx_sb[:, ch, :],
                            start=(ch == 0),
                            stop=(ch == NCH - 1),
                        )
                    nc.vector.tensor_copy(colsum_sb, pcs)

            # ================= attention, chunk-streamed =================
            with (
                tc.tile_pool(name="psnum", bufs=1, space="PSUM") as psnum,
                tc.tile_pool(name="psa", bufs=2, space="PSUM") as psa,
                tc.tile_pool(name="psfin", bufs=1, space="PSUM") as psfin,
                tc.tile_pool(name="pso", bufs=1, space="PSUM") as pso,
            ):
                nums = [
                    psnum.tile([65, L], F32, tag=f"pN{h}", name=f"num{h}")
                    for h in range(2)
                ]

                def chunk_scores(j):
                    """Scores/num contributions of kv-chunk j to t >= j*128,
                    plus the +1/2 causal terms for t-chunk j (after which
                    num[:, j*128:(j+1)*128] is complete)."""
                    tlo = j * 128
                    for h in range(2):
                        sq = sqp.tile(
                            [128, 1024], DT_ATT, tag="sq", name=f"sq{j}_{h}"
                        )
                        for a, b in _bank_splits(tlo, L):
                            w = b - a
                            pa = psa.tile(
                                [128, 512], F32, tag="pA", name=f"pa{j}_{h}_{a}"
                            )[:, :w]
                            nc.tensor.matmul(
                                pa,
                                kq_sb[32 * h : 32 * h + 32, tlo : tlo + 128],
                                kq_sb[32 * h : 32 * h + 32, 1024 + a : 1024 + b],
                                start=True,
                                stop=True,
                            )
                            sqs = sq[:, a - tlo : b - tlo]
                            nc.scalar.activation(
                                out=sqs,
                                in_=pa,
                                func=mybir.ActivationFunctionType.Square,
                                scale=A_SCALE,
                                bias=bias_sb,
                            )
                            if a == tlo:
                                # mask the diagonal block (i == j)
                                nc.vector.tensor_mul(
                                    sqs[:, 0:128], sqs[:, 0:128], tri_sb
                                )
                            # num^T += V_j^T-stationary @ sq
                            nc.tensor.matmul(
                                nums[h][:, a:b],
                                vx_sb[:, j, 65 * h : 65 * h + 65],
                                sqs,
                                start=(j == 0),
                                stop=False,
                            )
                        # intra-chunk +1/2 term: 0.5 * prefix-sums of V_j
                        nc.tensor.matmul(
                            nums[h][:, tlo : tlo + 128],
                            vx_sb[:, j, 65 * h : 65 * h + 65],
                            htri_sb,
                            start=False,
                            stop=False,
                        )
                        # inter-chunk +1/2 term: 0.5 * sum of prior colsums
                        nc.tensor.matmul(
                            nums[h][:, tlo : tlo + 128],
                            colsum_sb[:, 65 * h : 65 * h + 65],
                            sel_sb[:, tlo : tlo + 128],
                            start=False,
                            stop=True,
                        )

                def chunk_finalize(i):
                    """num[:, chunk i] is complete: divide by den and project
                    through Wo, stream the chunk to DRAM."""
                    tlo = i * 128
                    yT = []
                    for h in range(2):
                        # approx reciprocal needs base partition 0: run it
                        # over the whole [65,128] block, only row 64 is used
                        rc = wrk.tile([65, 128], F32, tag="rc")
                        nc.vector.reciprocal_approx_fast(
                            out=rc, in_=nums[h][:, tlo : tlo + 128]
                        )
                        prb = psfin.tile([64, 128], F32, tag="prb", name=f"prb{i}_{h}")
                        nc.tensor.matmul(
                            prb,
                            ones64_sb[64:65, :],
                            rc[64:65, :],
                            start=True,
                            stop=True,
                        )
                        rb = wrk.tile([64, 128], F32, tag="rb")
                        nc.any.tensor_copy(rb, prb)
                        y = wrk.tile([64, 128], DT_OUT, tag="yT")
                        nc.vector.tensor_mul(y, nums[h][0:64, tlo : tlo + 128], rb)
                        yT.append(y)
                    osb = wrk.tile([128, D], DT_OUT, tag="osb")
                    for a, b in ((0, 512), (512, 768)):
                        po = pso.tile([128, 512], F32, tag="po", name=f"po{i}_{a}")[
                            :, : b - a
                        ]
                        for h in range(2):
                            nc.tensor.matmul(
                                po,
                                yT[h],
                                wo_sb[h][:, a:b],
                                start=(h == 0),
                                stop=(h == 1),
                            )
                        nc.any.tensor_copy(osb[:, a:b], po)
                    nc.sync.dma_start(out=out.ap()[tlo : tlo + 128, :], in_=osb)

                # software pipeline: finalize chunk j-1 while chunk j's
                # scores run, so the PE never head-blocks on DVE work
                for j in range(NCH):
                    chunk_scores(j)
                    if j >= 1:
                        chunk_finalize(j - 1)
                chunk_finalize(NCH - 1)

    nc.finalize()
    return nc


def _host_consts():
    s = np.arange(128)[:, None]
    t = np.arange(128)[None, :]
    tri = (s <= t).astype(np.float32)
    htri = 0.5 * tri
    sel = np.zeros((8, 1024), dtype=np.float32)
    for i in range(8):
        sel[:i, i * 128 : (i + 1) * 128] = 0.5
    ones8 = np.zeros((128, 64), dtype=np.float32)
    for ch in range(8):
        ones8[:, ch * 8 + ch] = 1.0
    return tri, htri, sel, ones8


def kernel(hidden_states, Wq, Wk, Wv, Wo):
    global _compiled_nc, _last_in_maps
    hs = np.asarray(hidden_states, dtype=np.float32)[0]  # [L, D]
    Wq = np.asarray(Wq, dtype=np.float32)
    Wk = np.asarray(Wk, dtype=np.float32)
    Wv = np.asarray(Wv, dtype=np.float32)
    Wo = np.asarray(Wo, dtype=np.float32)

    if _compiled_nc is None:
        _compiled_nc = _build_nc()
    nc = _compiled_nc

    proj_dt = _np_dt(DT_PROJ)
    att_dt = _np_dt(DT_ATT)
    out_dt = _np_dt(DT_OUT)

    hsT = np.ascontiguousarray(hs.T).astype(proj_dt)  # [D, L]
    tri, htri, sel, ones8 = _host_consts()
    c_all = np.zeros((128, 1344), dtype=np.float32)
    c_all[:, 0:128] = tri
    c_all[:, 128:256] = htri
    c_all[:, 256:320] = ones8
    c_all[0:8, 320:1344] = sel
    c_all = c_all.astype(att_dt)

    in_maps = []
    for c in range(NCORE):
        heads = [2 * c, 2 * c + 1]
        wk_c = np.zeros((D, 64), dtype=np.float32)
        wq_c = np.zeros((D, 64), dtype=np.float32)
        wv_c = np.zeros((D, 130), dtype=np.float32)
        wo_c = np.zeros((128, D), dtype=np.float32)
        for hi, h in enumerate(heads):
            if h >= H:
                continue
            wk_c[:, 32 * hi : 32 * hi + FD] = Wk[:, h * FD : (h + 1) * FD]
            wq_c[:, 32 * hi : 32 * hi + FD] = Wq[:, h * FD : (h + 1) * FD]
            wv_c[:, 65 * hi : 65 * hi + HD] = Wv[:, h * HD : (h + 1) * HD]
            wo_c[64 * hi : 64 * hi + HD, :] = Wo[h * HD : (h + 1) * HD, :]
        wqv_c = np.concatenate([wk_c, wq_c, wv_c], axis=1)
        in_maps.append(
            {
                "hsT": hsT,
                "wqv": wqv_c.astype(proj_dt),
                "wo": wo_c.astype(out_dt),
                "c_all": c_all,
            }
        )

    _last_in_maps = in_maps
    res = run_bass_kernel_spmd(nc, in_maps, list(range(NCORE)))
    acc = np.zeros((L, D), dtype=np.float32)
    for c in range(NCORE):
        acc += np.asarray(res.results[c]["out"], dtype=np.float32)
    return acc.reshape(1, L, D)
